# revision 33
# baseline (speedup 1.0000x reference)
"""Distributed causal attention (qkv proj + RoPE + SDPA + out proj) on 8 trn2 cores.

Sharding: data-parallel over batch (B=2), tensor-parallel over heads
(12 heads -> 4 groups of 3). Core c handles batch c//4, heads 3*(c%4)..3*(c%4)+2.
Each core computes a partial output x_b @ Wqkv_heads -> attention -> @ Wo_rows;
the host sums the 4 head-group partials per batch.

Device layout per core (bf16 matmul operands, fp32 PSUM accumulation):
  xT    [768, 2048]  x[b] transposed (C-major), bf16
  wqkv  [768, 576]   columns: [q0 q1 | k0 k1 | q2 k2 | v0 v1 v2] (64 each), bf16
  wo    [192, 768]   Wo rows for the 3 heads, bf16
  cosT/sinT [128, 2048] RoPE tables, bf16 (row r -> head-dim r%64; sinT is
  row-swapped + sign-folded so swap-muls read in0/in1 at the same base partition)
  out   [2048, 768]  fp32 partial (pre-reduction) output

Attention is a lag-1 software pipeline over units
  [(01,0), (2,0), (01,1), (2,1), ...]:
PE runs the scores matmuls of unit i+1 while ACT exponentiates unit i, then PE's
PV matmuls of unit i follow.  Unit (01,j) computes heads 0 and 1 together with
row-packed K=64 matmuls (tile rows 0-63 / 64-127 concurrently); unit (2,j) does
head 2 alone, alternating row halves via duplicated q2/k2 so consecutive
LDWEIGHTS/matmuls overlap on the PE array.  After both units of query-block j
are normalized, that block's output projection + DMA-out run, spreading the
output tail across the whole phase.
"""
import numpy as np

B, T, C = 2, 2048, 768
H, DH = 12, 64
HPC = 3            # heads per core
NC_ = 8            # cores
QB = 512           # query block
KC = 128           # key chunk
NJ = T // QB       # 4 query blocks
NKC = T // KC      # 16 key chunks
SCALE = 1.0 / float(np.sqrt(DH))

_prog = None


def _build():
    import concourse.bass as bass
    import concourse.tile as tile
    from concourse import bacc, mybir

    f32 = mybir.dt.float32
    f32r = mybir.dt.float32r
    bf16 = mybir.dt.bfloat16
    Exp = mybir.ActivationFunctionType.Exp

    nc = bacc.Bacc("TRN2", target_bir_lowering=False, debug=False)

    xT_p = nc.declare_dram_parameter("xT", [C, T], bf16, isOutput=False)
    wqkv_p = nc.declare_dram_parameter("wqkv", [C, 576], bf16, isOutput=False)
    wo_p = nc.declare_dram_parameter("wo", [HPC * DH, C], bf16, isOutput=False)
    cos_p = nc.declare_dram_parameter("cosT", [128, T], bf16, isOutput=False)
    sin_p = nc.declare_dram_parameter("sinT", [128, T], bf16, isOutput=False)
    out_p = nc.declare_dram_parameter("out", [T, C], f32, isOutput=True)
    # DRAM bounce for the softmax-reciprocal partition-broadcast (SBUF APs
    # cannot have a zero partition step; DRAM APs can)
    recd_d = nc.dram_tensor("recd_dram", [1, HPC * NJ * QB], f32)

    with tile.TileContext(nc) as tc:
        with tc.tile_pool(name="persist", bufs=1) as persist:
            q01 = persist.tile([128, T], bf16, tag="q01")
            k01 = persist.tile([128, T], bf16, tag="k01")
            qk2 = persist.tile([128, T], bf16, tag="qk2")   # rows 0:64 q2, 64:128 q2 dup
            k2al = persist.tile([128, T], bf16, tag="k2al")  # rows 0:64 k2, 64:128 k2 dup
            vones = persist.tile([128, NKC, HPC, DH + 1], bf16, tag="vones")
            warm = persist.tile([1, 8], f32, tag="warm")

            # preload the exp table set while DMAs run
            nc.vector.memset(warm, 0.0)
            nc.scalar.activation(out=warm, in_=warm, func=Exp, scale=1.0)
            # ones column of vones (for the fused softmax denominator)
            nc.gpsimd.memset(vones[:, :, :, DH:DH + 1], 1.0)

            with tc.tile_pool(name="phaseA", bufs=1) as pa, \
                 tc.tile_pool(name="pp", bufs=1, space="PSUM") as pp, \
                 tc.tile_pool(name="vp", bufs=2, space="PSUM") as vp:
                wq = pa.tile([128, 6, 576], bf16, tag="wq")
                for k in range(6):
                    nc.sync.dma_start(
                        out=wq[:, k, :], in_=wqkv_p[k * 128:(k + 1) * 128, :])
                xts = []
                for k in range(6):
                    xt = pa.tile([128, T], bf16, tag=f"xt{k}")
                    # halves so the first projection matmuls start sooner
                    nc.sync.dma_start(out=xt[:, 0:T // 2],
                                      in_=xT_p[k * 128:(k + 1) * 128, 0:T // 2])
                    nc.sync.dma_start(out=xt[:, T // 2:T],
                                      in_=xT_p[k * 128:(k + 1) * 128, T // 2:T])
                    xts.append(xt)
                cosT = pa.tile([128, T], bf16, tag="cosT")
                sinT = pa.tile([128, T], bf16, tag="sinT")
                nc.sync.dma_start(out=cosT, in_=cos_p[:])
                nc.sync.dma_start(out=sinT, in_=sin_p[:])

                def rope(X, out_q, out_k):
                    """RoPE X (2 blocks of 64 rows) in place, except that rows
                    64:128 may be redirected to out_k (for q2k2 -> k2al)."""
                    tmp = pa.tile([128, T], bf16, tag="ropetmp")
                    nc.vector.tensor_mul(tmp[0:32], X[32:64], sinT[32:64])
                    nc.vector.tensor_mul(tmp[32:64], X[0:32], sinT[0:32])
                    nc.vector.tensor_mul(tmp[64:96], X[96:128], sinT[96:128])
                    nc.vector.tensor_mul(tmp[96:128], X[64:96], sinT[64:96])
                    nc.vector.tensor_mul(X, X, cosT)
                    if out_k is None:
                        nc.vector.tensor_add(X, X, tmp)
                    else:
                        nc.vector.tensor_add(out_q[0:64], X[0:64], tmp[0:64])
                        nc.vector.tensor_add(out_k[0:64], X[64:128], tmp[64:128])

                # q/k projection: M-tile m of qkvT = wqkv cols [128m, 128m+128)
                def proj_qk(m, X):
                    pst = []
                    for n in range(NJ):
                        ps = pp.tile([128, QB], f32, tag=f"pp{n}")
                        pst.append(ps)
                    for k in range(6):
                        for n in range(NJ):
                            nc.tensor.matmul(
                                pst[n],
                                lhsT=wq[:, k, m * 128:(m + 1) * 128],
                                rhs=xts[k][:, n * QB:(n + 1) * QB],
                                start=(k == 0), stop=(k == 5))
                    for n in range(NJ):
                        nc.scalar.copy(X[:, n * QB:(n + 1) * QB], pst[n])

                proj_qk(0, q01)
                rope(q01, None, None)
                proj_qk(1, k01)
                rope(k01, None, None)
                proj_qk(2, qk2)
                rope(qk2, qk2, k2al)
                # duplicate q2/k2 into rows 64:128 so head-2 score matmuls can
                # alternate PE row halves (LDWEIGHTS/matmul overlap)
                nc.vector.tensor_copy(qk2[64:128], qk2[0:64])
                nc.vector.tensor_copy(k2al[64:128], k2al[0:64])

                # v projection last: PE stays busy while DVE finishes RoPE
                for t in range(NKC):
                    ps = vp.tile([128, 192], f32, tag="vp")
                    for k in range(6):
                        nc.tensor.matmul(
                            ps, lhsT=xts[k][:, t * 128:(t + 1) * 128],
                            rhs=wq[:, k, 384:576],
                            start=(k == 0), stop=(k == 5))
                    nc.scalar.copy(
                        vones[:, t, :, 0:DH],
                        ps.rearrange("p (h d) -> p h d", h=HPC))

            # --- attention + per-block output projection ---
            with tc.tile_pool(name="phaseB", bufs=1) as pb, \
                 tc.tile_pool(name="rec", bufs=2) as rcp, \
                 tc.tile_pool(name="bct", bufs=2) as bcp, \
                 tc.tile_pool(name="ostage", bufs=3) as osp:
                # expt[:, hh, c, :] = exp of chunk c for head-slot hh.
                # Allocated first: they land on the SBUF freed by wq/xts (whose
                # readers finish early), not on the RoPE tables/scratch.
                expts = [pb.tile([128, 2, NKC, QB], bf16, name=f"expt{i}", tag=f"expt{i}")
                         for i in range(2)]
                outt01 = pb.tile([128, T], bf16, tag="outt01")
                outt2 = pb.tile([64, T], bf16, tag="outt2")
                # denominators, row r = j*HPC + h (unit-contiguous); recd = 1/denom
                denom = pb.tile([1, HPC * NJ * QB], f32, tag="denom")
                recd = pb.tile([1, HPC * NJ * QB], f32, tag="recd")
                wo01 = pb.tile([128, C], bf16, tag="wo01")
                nc.sync.dma_start(out=wo01, in_=wo_p[0:128, :])
                wo2 = pb.tile([64, C], bf16, tag="wo2")
                nc.sync.dma_start(out=wo2, in_=wo_p[128:192, :])

                def tgt_of(h):
                    return outt01[0:64] if h == 0 else (outt01[64:128] if h == 1 else outt2[0:64])

                with tc.tile_pool(name="sc", bufs=1, space="PSUM") as scp, \
                     tc.tile_pool(name="pv", bufs=2, space="PSUM") as pvp, \
                     tc.tile_pool(name="wp", bufs=1, space="PSUM") as wpp:

                    def s_steps(unit, expt):
                        """Closures: one per scores psum group (4 MMs + exp),
                        plus a final DVE causal-mask step."""
                        hh, j = unit
                        qsl = slice(j * QB, (j + 1) * QB)
                        steps = []
                        if hh == "01":
                            # heads 0+1 row-packed: per sc tile, 2 chunks each
                            def grp01(g):
                                sc = scp.tile([128, 4 * QB], f32, tag="sc",
                                              name=f"sc01_{j}_{g}")
                                for cc in range(2):
                                    c = 2 * g + cc
                                    # h0 rows 0:63, h1 rows 64:127 (concurrent)
                                    nc.tensor.matmul(
                                        sc[:, cc * QB:(cc + 1) * QB],
                                        lhsT=k01[0:64, c * KC:(c + 1) * KC],
                                        rhs=q01[0:64, qsl],
                                        start=True, stop=True)
                                    nc.tensor.matmul(
                                        sc[:, (2 + cc) * QB:(2 + cc + 1) * QB],
                                        lhsT=k01[64:128, c * KC:(c + 1) * KC],
                                        rhs=q01[64:128, qsl],
                                        start=True, stop=True)
                                nc.scalar.activation(
                                    out=expt[:, :, 2 * g:2 * g + 2, :],
                                    in_=sc.rearrange("p (hh cc q) -> p hh cc q", hh=2, cc=2),
                                    func=Exp, scale=SCALE)
                            for g in range(2 * (j + 1)):
                                steps.append(lambda g=g: grp01(g))

                            def mask01():
                                # zero exp entries above the causal diagonal:
                                # keep where q' - k - 128*u >= 0
                                nc.gpsimd.affine_select(
                                    out=expt[:, :, 4 * j:4 * j + 4, :],
                                    in_=expt[:, :, 4 * j:4 * j + 4, :],
                                    compare_op=mybir.AluOpType.is_ge, fill=0.0,
                                    base=0, pattern=[[0, 2], [-KC, 4], [1, QB]],
                                    channel_multiplier=-1)
                            steps.append(mask01)
                        else:
                            # head 2: alternate row halves for LDW/MM overlap
                            def grp2(g):
                                sc = scp.tile([128, 4 * QB], f32, tag="sc",
                                              name=f"sc2_{j}_{g}")
                                for u in range(4):
                                    c = 4 * g + u
                                    lo = u % 2 == 0
                                    nc.tensor.matmul(
                                        sc[:, u * QB:(u + 1) * QB],
                                        lhsT=k2al[0:64, c * KC:(c + 1) * KC] if lo
                                        else k2al[64:128, c * KC:(c + 1) * KC],
                                        rhs=qk2[0:64, qsl] if lo else qk2[64:128, qsl],
                                        start=True, stop=True)
                                nc.scalar.activation(
                                    out=expt[:, 0, 4 * g:4 * g + 4, :],
                                    in_=sc.rearrange("p (u q) -> p u q", u=4),
                                    func=Exp, scale=SCALE)
                            for g in range(j + 1):
                                steps.append(lambda g=g: grp2(g))

                            def mask2():
                                nc.gpsimd.affine_select(
                                    out=expt[:, 0, 4 * j:4 * j + 4, :],
                                    in_=expt[:, 0, 4 * j:4 * j + 4, :],
                                    compare_op=mybir.AluOpType.is_ge, fill=0.0,
                                    base=0, pattern=[[-KC, 4], [1, QB]],
                                    channel_multiplier=-1)
                            steps.append(mask2)
                        return steps

                    def p_steps(unit, expt):
                        """Closures: PV matmul chunk-steps, then copy+normalize,
                        then (after the '2' unit) the block's output projection."""
                        hh, j = unit
                        nch = 4 * (j + 1)
                        heads = [(0, 0), (1, 1)] if hh == "01" else [(2, 0)]
                        pos = {}
                        steps = []

                        def setup():
                            for h, _ in heads:
                                pos[h] = pvp.tile([128, QB], f32, tag="pv",
                                                  name=f"po_{h}_{j}")

                        def chunk(c):
                            for h, hh_slot in heads:
                                nc.tensor.matmul(
                                    pos[h][0:DH + 1, :],
                                    lhsT=vones[:, c, h, :],
                                    rhs=expt[:, hh_slot, c, :],
                                    start=(c == 0), stop=(c == nch - 1))

                        steps.append(setup)
                        for c0 in range(0, nch, 2):
                            def two(c0=c0):
                                chunk(c0)
                                chunk(c0 + 1)
                            steps.append(two)

                        def fin(h, hh_slot):
                            po = pos[h]
                            tgt = tgt_of(h)
                            r = j * HPC + h
                            nc.vector.tensor_copy(
                                tgt[:, j * QB:(j + 1) * QB], po[0:DH, :])
                            nc.vector.tensor_copy(
                                denom[0:1, r * QB:(r + 1) * QB], po[DH:DH + 1, :])

                        def norm_unit():
                            # batched 1/denom for this unit's contiguous rows,
                            # then per-head partition-broadcast DMA + multiply
                            r0 = j * HPC + heads[0][0]
                            r1 = j * HPC + heads[-1][0]
                            usl = slice(r0 * QB, (r1 + 1) * QB)
                            with nc.allow_low_precision(reason="softmax denom reciprocal: 18-bit approx"):
                                nc.vector.reciprocal_approx_fast(
                                    out=recd[0:1, usl], in_=denom[0:1, usl])
                            nc.sync.dma_start(out=recd_d[0:1, usl],
                                              in_=recd[0:1, usl])
                            for h, _ in heads:
                                r = j * HPC + h
                                base = 64 if h == 1 else 0
                                src = recd_d[0:1, r * QB:(r + 1) * QB]
                                bsrc = bass.AP(
                                    tensor=src.tensor, offset=src.offset,
                                    ap=[[0, 64]] + list(src.ap[1:]))
                                bct = bcp.tile([128, QB], f32, tag="bct",
                                               name=f"bct_{h}_{j}")
                                nc.sync.dma_start(
                                    out=bct[base:base + 64, :], in_=bsrc)
                                tgt = tgt_of(h)
                                sl = slice(j * QB, (j + 1) * QB)
                                nc.vector.tensor_mul(
                                    tgt[:, sl], tgt[:, sl], bct[base:base + 64, :])

                        for h, hh_slot in heads:
                            steps.append(lambda h=h, s=hh_slot: fin(h, s))
                        steps.append(norm_unit)

                        if hh == "2":
                            def wo_tile(qq):
                                q = j * 4 + qq
                                pw = wpp.tile([128, 1024], f32, tag="wp",
                                              name=f"pw_{q}")
                                for (n0, n1) in ((0, 512), (512, 768)):
                                    nc.tensor.matmul(
                                        pw[:, n0:n1],
                                        lhsT=outt01[:, q * 128:(q + 1) * 128],
                                        rhs=wo01[:, n0:n1],
                                        start=True, stop=False)
                                    nc.tensor.matmul(
                                        pw[:, n0:n1],
                                        lhsT=outt2[:, q * 128:(q + 1) * 128],
                                        rhs=wo2[:, n0:n1],
                                        start=False, stop=True)
                                ot = osp.tile([128, C], f32, tag="ot",
                                              name=f"ot_{q}")
                                if qq % 2 == 0:
                                    nc.scalar.copy(ot, pw[:, 0:C])
                                else:
                                    nc.vector.tensor_copy(ot, pw[:, 0:C])
                                nc.sync.dma_start(
                                    out=out_p[q * 128:(q + 1) * 128, :], in_=ot)
                            for qq in range(4):
                                steps.append(lambda qq=qq: wo_tile(qq))
                        return steps

                    units = []
                    for j in range(NJ):
                        units.append(("01", j))
                        units.append(("2", j))

                    # lag-1 pipeline, interleaved at step granularity: PE runs
                    # the previous unit's PV/Wo steps in the gaps between this
                    # unit's score groups (which are paced by ACT's exp).
                    prev_p = []
                    for i, u in enumerate(units):
                        S = s_steps(u, expts[i % 2])
                        done = 0
                        for gi, s in enumerate(S):
                            s()
                            want = ((gi + 1) * len(prev_p)) // len(S)
                            while done < want:
                                prev_p[done]()
                                done += 1
                        while done < len(prev_p):
                            prev_p[done]()
                            done += 1
                        prev_p = p_steps(u, expts[i % 2])
                    for p in prev_p:
                        p()

    nc.compile()
    return nc


def _host_prep(x, Wqkv, Wo, seq_len):
    import ml_dtypes
    bf16 = ml_dtypes.bfloat16
    x = np.asarray(x, dtype=np.float32)
    Wqkv = np.asarray(Wqkv, dtype=np.float32)
    Wo = np.asarray(Wo, dtype=np.float32)
    off = int(np.asarray(seq_len).reshape(()))

    inv = 1.0 / (10000.0 ** (np.arange(0, DH, 2, dtype=np.float64) / DH))  # [32]
    pos = np.arange(T, dtype=np.float64) + off
    ang = pos[:, None] * inv[None, :]                 # [T, 32]
    cs = np.cos(ang).T                                # [32, T]
    sn = np.sin(ang).T
    cos128 = np.empty((128, T), np.float32)
    sin128 = np.empty((128, T), np.float32)
    for blk in range(2):
        r0 = blk * 64
        cos128[r0:r0 + 32] = cs
        cos128[r0 + 32:r0 + 64] = cs
        # row-swapped + sign-folded: row s holds the coefficient X[s] is
        # multiplied by when producing output row s^32 (see rope()).
        sin128[r0:r0 + 32] = sn
        sin128[r0 + 32:r0 + 64] = -sn

    in_maps = []
    for core in range(NC_):
        b, g = core // 4, core % 4
        hs = [3 * g, 3 * g + 1, 3 * g + 2]
        q = [Wqkv[:, h * DH:(h + 1) * DH] for h in hs]
        k = [Wqkv[:, C + h * DH:C + (h + 1) * DH] for h in hs]
        v = [Wqkv[:, 2 * C + h * DH:2 * C + (h + 1) * DH] for h in hs]
        wqkv_l = np.concatenate(
            [q[0], q[1], k[0], k[1], q[2], k[2], v[0], v[1], v[2]], axis=1)
        in_maps.append({
            "xT": np.ascontiguousarray(x[b].T).astype(bf16),
            "wqkv": np.ascontiguousarray(wqkv_l).astype(bf16),
            "wo": np.ascontiguousarray(
                Wo[g * HPC * DH:(g + 1) * HPC * DH, :]).astype(bf16),
            "cosT": cos128.astype(bf16),
            "sinT": sin128.astype(bf16),
        })
    return in_maps


def _run(in_maps, trace=False):
    global _prog
    from concourse.bass_utils import run_bass_kernel_spmd
    if _prog is None:
        _prog = _build()
    return run_bass_kernel_spmd(_prog, in_maps, list(range(NC_)), trace=trace)


def kernel(x, Wqkv, Wo, seq_len):
    in_maps = _host_prep(x, Wqkv, Wo, seq_len)
    res = _run(in_maps, trace=False)
    out = np.zeros((B, T, C), dtype=np.float32)
    for core in range(NC_):
        out[core // 4] += res.results[core]["out"]
    return out


# revision 39
# speedup vs baseline: 1.0932x; 1.0932x over previous
"""Distributed causal attention (qkv proj + RoPE + SDPA + out proj) on 8 trn2 cores.

Sharding: data-parallel over batch (B=2), tensor-parallel over heads
(12 heads -> 4 groups of 3). Core c handles batch c//4, heads 3*(c%4)..3*(c%4)+2.
Each core computes a partial output x_b @ Wqkv_heads -> attention -> @ Wo_rows;
the host sums the 4 head-group partials per batch.

Device layout per core (bf16 matmul operands, fp32 PSUM accumulation):
  xT    [768, 2048]  x[b] transposed (C-major), bf16
  wqkv  [768, 576]   columns: [q0 q1 | k0 k1 | q2 k2 | v0 v1 v2] (64 each), bf16
  wo    [192, 768]   Wo rows for the 3 heads, bf16
  cosT/sinT [128, 2048] RoPE tables, bf16 (row r -> head-dim r%64; sinT is
  row-swapped + sign-folded so swap-muls read in0/in1 at the same base partition)
  out   [2048, 768]  fp32 partial (pre-reduction) output

Attention is a lag-1 software pipeline over units
  [(01,0), (2,0), (01,1), (2,1), ...]:
PE runs the scores matmuls of unit i+1 while ACT exponentiates unit i, then PE's
PV matmuls of unit i follow.  Unit (01,j) computes heads 0 and 1 together with
row-packed K=64 matmuls (tile rows 0-63 / 64-127 concurrently); unit (2,j) does
head 2 alone, alternating row halves via duplicated q2/k2 so consecutive
LDWEIGHTS/matmuls overlap on the PE array.  After both units of query-block j
are normalized, that block's output projection + DMA-out run, spreading the
output tail across the whole phase.
"""
import numpy as np

B, T, C = 2, 2048, 768
H, DH = 12, 64
HPC = 3            # heads per core
NC_ = 8            # cores
QB = 512           # query block
KC = 128           # key chunk
NJ = T // QB       # 4 query blocks
NKC = T // KC      # 16 key chunks
SCALE = 1.0 / float(np.sqrt(DH))

_prog = None


def _build():
    import concourse.bass as bass
    import concourse.tile as tile
    from concourse import bacc, mybir

    f32 = mybir.dt.float32
    f32r = mybir.dt.float32r
    bf16 = mybir.dt.bfloat16
    Exp = mybir.ActivationFunctionType.Exp

    nc = bacc.Bacc("TRN2", target_bir_lowering=False, debug=False)

    xT_p = nc.declare_dram_parameter("xT", [C, T], bf16, isOutput=False)
    wqkv_p = nc.declare_dram_parameter("wqkv", [C, 576], bf16, isOutput=False)
    wo_p = nc.declare_dram_parameter("wo", [HPC * DH, C], bf16, isOutput=False)
    cos_p = nc.declare_dram_parameter("cosT", [128, T], bf16, isOutput=False)
    sin_p = nc.declare_dram_parameter("sinT", [128, T], bf16, isOutput=False)
    out_p = nc.declare_dram_parameter("out", [T, C], f32, isOutput=True)
    # DRAM bounce for the softmax-reciprocal partition-broadcast (SBUF APs
    # cannot have a zero partition step; DRAM APs can)
    recd_d = nc.dram_tensor("recd_dram", [1, HPC * NJ * QB], f32)

    with tile.TileContext(nc) as tc:
        with tc.tile_pool(name="persist", bufs=1) as persist:
            q01 = persist.tile([128, T], bf16, tag="q01")
            k01 = persist.tile([128, T], bf16, tag="k01")
            qk2 = persist.tile([128, T], bf16, tag="qk2")   # rows 0:64 q2, 64:128 q2 dup
            k2al = persist.tile([128, T], bf16, tag="k2al")  # rows 0:64 k2, 64:128 k2 dup
            vones = persist.tile([128, NKC, HPC, DH + 1], bf16, tag="vones")
            mask = persist.tile([128, 4, QB], bf16, tag="mask")
            warm = persist.tile([1, 8], f32, tag="warm")

            # preload the exp table set while DMAs run
            nc.vector.memset(warm, 0.0)
            nc.scalar.activation(out=warm, in_=warm, func=Exp, scale=1.0)
            # causal mask for the diagonal 4-chunk group:
            # mask[k, u, q'] = 1 if q' >= k + 128*u else 0
            nc.gpsimd.memset(mask, 1.0)
            nc.gpsimd.affine_select(
                out=mask, in_=mask,
                compare_op=mybir.AluOpType.is_ge, fill=0.0, base=0,
                pattern=[[-KC, 4], [1, QB]], channel_multiplier=-1,
            )
            # ones column of vones (for the fused softmax denominator)
            nc.gpsimd.memset(vones[:, :, :, DH:DH + 1], 1.0)

            with tc.tile_pool(name="phaseA", bufs=1) as pa, \
                 tc.tile_pool(name="pp", bufs=1, space="PSUM") as pp, \
                 tc.tile_pool(name="vp", bufs=2, space="PSUM") as vp:
                wq = pa.tile([128, 6, 576], bf16, tag="wq")
                for k in range(6):
                    nc.sync.dma_start(
                        out=wq[:, k, :], in_=wqkv_p[k * 128:(k + 1) * 128, :])
                xts = []
                for k in range(6):
                    xt = pa.tile([128, T], bf16, tag=f"xt{k}", name=f"xt{k}")
                    xts.append(xt)
                # first halves of every chunk land before any second half, so
                # the m0/m1 projections (n-blocks 0-1) start ~2x sooner
                for half in range(2):
                    hs = slice(half * (T // 2), (half + 1) * (T // 2))
                    for k in range(6):
                        nc.sync.dma_start(out=xts[k][:, hs],
                                          in_=xT_p[k * 128:(k + 1) * 128, hs])
                cosT = pa.tile([128, T], bf16, tag="cosT")
                sinT = pa.tile([128, T], bf16, tag="sinT")
                nc.sync.dma_start(out=cosT, in_=cos_p[:])
                nc.sync.dma_start(out=sinT, in_=sin_p[:])

                def rope(X, out_q, out_k, sl):
                    """RoPE X[:, sl] (2 blocks of 64 rows) in place, except
                    that rows 64:128 may go to out_k (for q2k2 -> k2al)."""
                    tmp = pa.tile([128, T // 2], bf16, tag="ropetmp",
                                  name="ropetmp")
                    w = sl.stop - sl.start
                    tp = tmp[:, 0:w]
                    nc.vector.tensor_mul(tp[0:32], X[32:64, sl], sinT[32:64, sl])
                    nc.vector.tensor_mul(tp[32:64], X[0:32, sl], sinT[0:32, sl])
                    nc.vector.tensor_mul(tp[64:96], X[96:128, sl], sinT[96:128, sl])
                    nc.vector.tensor_mul(tp[96:128], X[64:96, sl], sinT[64:96, sl])
                    nc.vector.tensor_mul(X[:, sl], X[:, sl], cosT[:, sl])
                    if out_k is None:
                        nc.vector.tensor_add(X[:, sl], X[:, sl], tp)
                    else:
                        nc.vector.tensor_add(out_q[0:64, sl], X[0:64, sl], tp[0:64])
                        nc.vector.tensor_add(out_k[0:64, sl], X[64:128, sl], tp[64:128])

                # q/k projection: M-tile m of qkvT = wqkv cols [128m, 128m+128),
                # pipelined per T-half so copies + RoPE overlap the matmuls
                def proj_qk(m, X, out_q=None, out_k=None):
                    for half in range(2):
                        pst = []
                        for nn in range(2):
                            n = half * 2 + nn
                            ps = pp.tile([128, QB], f32, tag=f"pp{nn}",
                                         name=f"pp{m}_{n}")
                            pst.append(ps)
                        for k in range(6):
                            for nn in range(2):
                                n = half * 2 + nn
                                nc.tensor.matmul(
                                    pst[nn],
                                    lhsT=wq[:, k, m * 128:(m + 1) * 128],
                                    rhs=xts[k][:, n * QB:(n + 1) * QB],
                                    start=(k == 0), stop=(k == 5))
                        for nn in range(2):
                            n = half * 2 + nn
                            nc.scalar.copy(X[:, n * QB:(n + 1) * QB], pst[nn])
                        rope(X, out_q, out_k,
                             slice(half * (T // 2), (half + 1) * (T // 2)))

                proj_qk(0, q01)
                proj_qk(1, k01)
                proj_qk(2, qk2, out_q=qk2, out_k=k2al)
                # duplicate q2/k2 into rows 64:128 so head-2 score matmuls can
                # alternate PE row halves (LDWEIGHTS/matmul overlap)
                nc.vector.tensor_copy(qk2[64:128], qk2[0:64])
                nc.vector.tensor_copy(k2al[64:128], k2al[0:64])

                # v projection last: PE stays busy while DVE finishes RoPE
                for t in range(NKC):
                    ps = vp.tile([128, 192], f32, tag="vp")
                    for k in range(6):
                        nc.tensor.matmul(
                            ps, lhsT=xts[k][:, t * 128:(t + 1) * 128],
                            rhs=wq[:, k, 384:576],
                            start=(k == 0), stop=(k == 5))
                    nc.scalar.copy(
                        vones[:, t, :, 0:DH],
                        ps.rearrange("p (h d) -> p h d", h=HPC))

            # --- attention + per-block output projection ---
            with tc.tile_pool(name="phaseB", bufs=1) as pb, \
                 tc.tile_pool(name="rec", bufs=2) as rcp, \
                 tc.tile_pool(name="bct", bufs=2) as bcp, \
                 tc.tile_pool(name="ostage", bufs=3) as osp:
                # expt[:, hh, c, :] = exp of chunk c for head-slot hh.
                # Allocated first: they land on the SBUF freed by wq/xts (whose
                # readers finish early), not on the RoPE tables/scratch.
                expts = [pb.tile([128, 2, NKC, QB], bf16, name=f"expt{i}", tag=f"expt{i}")
                         for i in range(2)]
                outt01 = pb.tile([128, T], bf16, tag="outt01")
                outt2 = pb.tile([64, T], bf16, tag="outt2")
                # denominators, row r = j*HPC + h (unit-contiguous); recd = 1/denom
                denom = pb.tile([1, HPC * NJ * QB], f32, tag="denom")
                recd = pb.tile([1, HPC * NJ * QB], f32, tag="recd")
                wo01 = pb.tile([128, C], bf16, tag="wo01")
                nc.sync.dma_start(out=wo01, in_=wo_p[0:128, :])
                wo2 = pb.tile([64, C], bf16, tag="wo2")
                nc.sync.dma_start(out=wo2, in_=wo_p[128:192, :])

                def tgt_of(h):
                    return outt01[0:64] if h == 0 else (outt01[64:128] if h == 1 else outt2[0:64])

                with tc.tile_pool(name="sc", bufs=1, space="PSUM") as scp, \
                     tc.tile_pool(name="pv", bufs=2, space="PSUM") as pvp, \
                     tc.tile_pool(name="wp", bufs=1, space="PSUM") as wpp:

                    def s_steps(unit, expt):
                        """Closures: one per scores psum group (4 MMs + exp),
                        plus a final DVE causal-mask step."""
                        hh, j = unit
                        qsl = slice(j * QB, (j + 1) * QB)
                        steps = []
                        if hh == "01":
                            # heads 0+1 row-packed: per sc tile, 2 chunks each
                            def grp01(g):
                                sc = scp.tile([128, 4 * QB], f32, tag="sc",
                                              name=f"sc01_{j}_{g}")
                                for cc in range(2):
                                    c = 2 * g + cc
                                    # h0 rows 0:63, h1 rows 64:127 (concurrent)
                                    nc.tensor.matmul(
                                        sc[:, cc * QB:(cc + 1) * QB],
                                        lhsT=k01[0:64, c * KC:(c + 1) * KC],
                                        rhs=q01[0:64, qsl],
                                        start=True, stop=True)
                                    nc.tensor.matmul(
                                        sc[:, (2 + cc) * QB:(2 + cc + 1) * QB],
                                        lhsT=k01[64:128, c * KC:(c + 1) * KC],
                                        rhs=q01[64:128, qsl],
                                        start=True, stop=True)
                                nc.scalar.activation(
                                    out=expt[:, :, 2 * g:2 * g + 2, :],
                                    in_=sc.rearrange("p (hh cc q) -> p hh cc q", hh=2, cc=2),
                                    func=Exp, scale=SCALE)
                            def m01(g):
                                # zero exp entries above the causal diagonal of
                                # this group's 2 chunks (offsets u, u+1)
                                u = 2 * g - 4 * j
                                for hh_ in range(2):
                                    nc.vector.tensor_mul(
                                        expt[:, hh_, 2 * g:2 * g + 2, :],
                                        expt[:, hh_, 2 * g:2 * g + 2, :],
                                        mask[:, u:u + 2, :])
                            for g in range(2 * (j + 1)):
                                if g >= 2 * j:
                                    steps.append(lambda g=g: (grp01(g), m01(g)))
                                else:
                                    steps.append(lambda g=g: grp01(g))
                        else:
                            # head 2: alternate row halves for LDW/MM overlap
                            def grp2(g):
                                sc = scp.tile([128, 4 * QB], f32, tag="sc",
                                              name=f"sc2_{j}_{g}")
                                for u in range(4):
                                    c = 4 * g + u
                                    lo = u % 2 == 0
                                    nc.tensor.matmul(
                                        sc[:, u * QB:(u + 1) * QB],
                                        lhsT=k2al[0:64, c * KC:(c + 1) * KC] if lo
                                        else k2al[64:128, c * KC:(c + 1) * KC],
                                        rhs=qk2[0:64, qsl] if lo else qk2[64:128, qsl],
                                        start=True, stop=True)
                                nc.scalar.activation(
                                    out=expt[:, 0, 4 * g:4 * g + 4, :],
                                    in_=sc.rearrange("p (u q) -> p u q", u=4),
                                    func=Exp, scale=SCALE)
                            def m2():
                                nc.vector.tensor_mul(
                                    expt[:, 0, 4 * j:4 * j + 4, :],
                                    expt[:, 0, 4 * j:4 * j + 4, :], mask)
                            for g in range(j + 1):
                                if g == j:
                                    steps.append(lambda g=g: (grp2(g), m2()))
                                else:
                                    steps.append(lambda g=g: grp2(g))
                        return steps

                    def p_steps(unit, expt):
                        """Closures: PV matmul chunk-steps, then copy+normalize,
                        then (after the '2' unit) the block's output projection."""
                        hh, j = unit
                        nch = 4 * (j + 1)
                        heads = [(0, 0), (1, 1)] if hh == "01" else [(2, 0)]
                        pos = {}
                        steps = []

                        def setup():
                            for h, _ in heads:
                                pos[h] = pvp.tile([128, QB], f32, tag="pv",
                                                  name=f"po_{h}_{j}")

                        def chunk(c):
                            for h, hh_slot in heads:
                                nc.tensor.matmul(
                                    pos[h][0:DH + 1, :],
                                    lhsT=vones[:, c, h, :],
                                    rhs=expt[:, hh_slot, c, :],
                                    start=(c == 0), stop=(c == nch - 1))

                        steps.append(setup)
                        for c0 in range(0, nch, 2):
                            def two(c0=c0):
                                chunk(c0)
                                chunk(c0 + 1)
                            steps.append(two)

                        def fin(h, hh_slot):
                            po = pos[h]
                            tgt = tgt_of(h)
                            r = j * HPC + h
                            nc.vector.tensor_copy(
                                tgt[:, j * QB:(j + 1) * QB], po[0:DH, :])
                            nc.vector.tensor_copy(
                                denom[0:1, r * QB:(r + 1) * QB], po[DH:DH + 1, :])

                        def norm_unit():
                            # batched 1/denom for this unit's contiguous rows,
                            # then per-head partition-broadcast DMA + multiply
                            r0 = j * HPC + heads[0][0]
                            r1 = j * HPC + heads[-1][0]
                            usl = slice(r0 * QB, (r1 + 1) * QB)
                            with nc.allow_low_precision(reason="softmax denom reciprocal: 18-bit approx"):
                                nc.vector.reciprocal_approx_fast(
                                    out=recd[0:1, usl], in_=denom[0:1, usl])
                            nc.sync.dma_start(out=recd_d[0:1, usl],
                                              in_=recd[0:1, usl])
                            for h, _ in heads:
                                r = j * HPC + h
                                base = 64 if h == 1 else 0
                                src = recd_d[0:1, r * QB:(r + 1) * QB]
                                bsrc = bass.AP(
                                    tensor=src.tensor, offset=src.offset,
                                    ap=[[0, 64]] + list(src.ap[1:]))
                                bct = bcp.tile([128, QB], f32, tag="bct",
                                               name=f"bct_{h}_{j}")
                                nc.sync.dma_start(
                                    out=bct[base:base + 64, :], in_=bsrc)
                                tgt = tgt_of(h)
                                sl = slice(j * QB, (j + 1) * QB)
                                nc.vector.tensor_mul(
                                    tgt[:, sl], tgt[:, sl], bct[base:base + 64, :])

                        for h, hh_slot in heads:
                            steps.append(lambda h=h, s=hh_slot: fin(h, s))
                        steps.append(norm_unit)

                        if hh == "2":
                            pws = {}

                            def wo_mm(qq):
                                q = j * 4 + qq
                                pw = wpp.tile([128, 1024], f32, tag="wp",
                                              name=f"pw_{q}")
                                pws[qq] = pw
                                for (n0, n1) in ((0, 512), (512, 768)):
                                    nc.tensor.matmul(
                                        pw[:, n0:n1],
                                        lhsT=outt01[:, q * 128:(q + 1) * 128],
                                        rhs=wo01[:, n0:n1],
                                        start=True, stop=False)
                                    nc.tensor.matmul(
                                        pw[:, n0:n1],
                                        lhsT=outt2[:, q * 128:(q + 1) * 128],
                                        rhs=wo2[:, n0:n1],
                                        start=False, stop=True)

                            def wo_out(qq):
                                q = j * 4 + qq
                                pw = pws[qq]
                                ot = osp.tile([128, C], f32, tag="ot",
                                              name=f"ot_{q}")
                                if qq % 2 == 0:
                                    nc.scalar.copy(ot, pw[:, 0:C])
                                else:
                                    nc.vector.tensor_copy(ot, pw[:, 0:C])
                                nc.sync.dma_start(
                                    out=out_p[q * 128:(q + 1) * 128, :], in_=ot)
                            for qq in range(4):
                                steps.append(lambda qq=qq: wo_mm(qq))
                                steps.append(lambda qq=qq: wo_out(qq))
                        return steps

                    units = []
                    for j in range(NJ):
                        units.append(("01", j))
                        units.append(("2", j))

                    # lag-1 pipeline, interleaved at step granularity: PE runs
                    # the previous unit's PV/Wo steps in the gaps between this
                    # unit's score groups (which are paced by ACT's exp).
                    prev_p = []
                    for i, u in enumerate(units):
                        S = s_steps(u, expts[i % 2])
                        done = 0
                        for gi, s in enumerate(S):
                            s()
                            want = ((gi + 1) * len(prev_p)) // len(S)
                            while done < want:
                                prev_p[done]()
                                done += 1
                        while done < len(prev_p):
                            prev_p[done]()
                            done += 1
                        prev_p = p_steps(u, expts[i % 2])
                    for p in prev_p:
                        p()

    nc.compile()
    return nc


def _host_prep(x, Wqkv, Wo, seq_len):
    import ml_dtypes
    bf16 = ml_dtypes.bfloat16
    x = np.asarray(x, dtype=np.float32)
    Wqkv = np.asarray(Wqkv, dtype=np.float32)
    Wo = np.asarray(Wo, dtype=np.float32)
    off = int(np.asarray(seq_len).reshape(()))

    inv = 1.0 / (10000.0 ** (np.arange(0, DH, 2, dtype=np.float64) / DH))  # [32]
    pos = np.arange(T, dtype=np.float64) + off
    ang = pos[:, None] * inv[None, :]                 # [T, 32]
    cs = np.cos(ang).T                                # [32, T]
    sn = np.sin(ang).T
    cos128 = np.empty((128, T), np.float32)
    sin128 = np.empty((128, T), np.float32)
    for blk in range(2):
        r0 = blk * 64
        cos128[r0:r0 + 32] = cs
        cos128[r0 + 32:r0 + 64] = cs
        # row-swapped + sign-folded: row s holds the coefficient X[s] is
        # multiplied by when producing output row s^32 (see rope()).
        sin128[r0:r0 + 32] = sn
        sin128[r0 + 32:r0 + 64] = -sn

    in_maps = []
    for core in range(NC_):
        b, g = core // 4, core % 4
        hs = [3 * g, 3 * g + 1, 3 * g + 2]
        q = [Wqkv[:, h * DH:(h + 1) * DH] for h in hs]
        k = [Wqkv[:, C + h * DH:C + (h + 1) * DH] for h in hs]
        v = [Wqkv[:, 2 * C + h * DH:2 * C + (h + 1) * DH] for h in hs]
        wqkv_l = np.concatenate(
            [q[0], q[1], k[0], k[1], q[2], k[2], v[0], v[1], v[2]], axis=1)
        in_maps.append({
            "xT": np.ascontiguousarray(x[b].T).astype(bf16),
            "wqkv": np.ascontiguousarray(wqkv_l).astype(bf16),
            "wo": np.ascontiguousarray(
                Wo[g * HPC * DH:(g + 1) * HPC * DH, :]).astype(bf16),
            "cosT": cos128.astype(bf16),
            "sinT": sin128.astype(bf16),
        })
    return in_maps


def _run(in_maps, trace=False):
    global _prog
    from concourse.bass_utils import run_bass_kernel_spmd
    if _prog is None:
        _prog = _build()
    return run_bass_kernel_spmd(_prog, in_maps, list(range(NC_)), trace=trace)


def kernel(x, Wqkv, Wo, seq_len):
    in_maps = _host_prep(x, Wqkv, Wo, seq_len)
    res = _run(in_maps, trace=False)
    out = np.zeros((B, T, C), dtype=np.float32)
    for core in range(NC_):
        out[core // 4] += res.results[core]["out"]
    return out
